# revision 12
# baseline (speedup 1.0000x reference)
"""Trainium2 Bass kernel for DeepMultiBasisBilinearNet.

Strategy: pure data-parallel over the batch (16384/8 = 2048 rows per core).
All activations kept in transposed [D, B] layout on-device so that every
matmul contraction dim lands on SBUF partitions with zero on-device
transposes (host pre-transposes x and all weights). All matmuls run in
fp16 (fp32 PSUM accumulation). LayerNorm statistics are computed in
broadcast form: the 8 hpre chunks (and their squares) are tree-summed on
DVE, then ones/D stationary matmuls land the mean / mean-square on all
128 partitions at once, so the variance -> rstd chain and the two-op
apply run on full-width [128, NB] tiles with no PE involvement.

Block emissions are software-pipelined with a one-tile lag --
b1(0), b1(1), b2(0), b1(2), b2(1), ... -- so each layernorm chain+apply
(and each head) hides under an *independent* block's matmul stream; the
only exposed serial tail is the last tile's LN2+head.
"""

import sys

if "/opt/trn_rl_repo" not in sys.path:
    sys.path.insert(0, "/opt/trn_rl_repo")

import numpy as np

import concourse.bass as bass
import concourse.tile as tile
from concourse import bacc, mybir
from concourse.bass_utils import run_bass_kernel_spmd

F16 = mybir.dt.float16
F32 = mybir.dt.float32
AF = mybir.ActivationFunctionType
ALU = mybir.AluOpType

P = 128
B, D, H, R, OUT = 16384, 1024, 4, 1024, 10
HR = H * R                 # 4096
NCORES = 8
BC = B // NCORES           # 2048 rows per core
NB = 512                   # batch tile (matmul free dim, one PSUM bank)
DC = D // P                # 8 chunks of the model dim
JC = HR // P               # 32 chunks of the bilinear dim
EGRP = 2                   # eigen-projection dout groups (PSUM pressure)
EGS = DC // EGRP           # douts per group
LN_EPS = 1e-5


def _emit_block_matmuls(nc, pools, dram, blk, acts, t, tail=False):
    """Bilinear matmuls + residual + broadcast LN stats for batch tile t.

    acts: list of 8 SBUF fp16 tiles [128, NB] holding the input in [d, b]
    layout.  Returns (hpre chunk list, mu16, rstd16) for the LN apply.
    """
    sb, wp, wep_p, ip, pp, psr, pse, pst, cst = (
        pools["sb"], pools["wp"], pools["wep"], pools["ip"], pools["pp"],
        pools["ps_rl"], pools["ps_e"], pools["ps_st"], pools["const"],
    )
    br_sb = cst[f"br{blk}"]
    bl_sb = cst[f"bl{blk}"]
    be_sb = cst[f"be{blk}"]

    inter = []
    for jc in range(JC):
        wr_t = wp.tile([P, D], F16, tag="wrl")
        nc.sync.dma_start(out=wr_t[:], in_=dram[f"wr{blk}"][jc])
        wl_t = wp.tile([P, D], F16, tag="wrl")
        nc.sync.dma_start(out=wl_t[:], in_=dram[f"wl{blk}"][jc])

        ps_r = psr.tile([P, NB], F32, tag="rl")
        for dc in range(DC):
            nc.tensor.matmul(
                ps_r[:], wr_t[:, dc * P:(dc + 1) * P], acts[dc][:],
                start=(dc == 0), stop=(dc == DC - 1),
            )
        # evict right off PSUM immediately (ACT) so the bank frees fast
        tmp_r = pp.tile([P, NB], F16, tag="tmp_r")
        nc.scalar.activation(tmp_r[:], ps_r[:], AF.Identity,
                             bias=br_sb[:, jc:jc + 1])

        ps_l = psr.tile([P, NB], F32, tag="rl")
        for dc in range(DC):
            nc.tensor.matmul(
                ps_l[:], wl_t[:, dc * P:(dc + 1) * P], acts[dc][:],
                start=(dc == 0), stop=(dc == DC - 1),
            )
        tmp_l = pp.tile([P, NB], F16, tag="tmp_l")
        nc.scalar.activation(tmp_l[:], ps_l[:], AF.Identity,
                             bias=bl_sb[:, jc:jc + 1])

        it = ip.tile([P, NB], F16, tag="inter")
        nc.vector.tensor_mul(it[:], tmp_r[:], tmp_l[:])
        inter.append(it)

    # eigen projection: [D, NB] += weT[jc] @ inter[jc], in EGRP dout groups.
    # hpre chunks are pairwise tree-summed (values and squares) on DVE so
    # the broadcast LN stats need only one matmul per statistic.
    def _tree(nodes, tag):
        while len(nodes) > 1:
            nxt = []
            for i in range(0, len(nodes) - 1, 2):
                s = sb.tile([P, NB], F16, tag=tag, bufs=6)
                nc.vector.tensor_add(s[:], nodes[i][:], nodes[i + 1][:])
                nxt.append(s)
            if len(nodes) % 2:
                nxt.append(nodes[-1])
            nodes = nxt
        return nodes[0]

    # broadcast-form stats: ones/D stationary -> mean on all 128 partitions.
    # For the final (tail) block the stats matmuls are split into two
    # accumulation halves so half the reduction lands mid-eigen.
    ones_dd = cst["ones_dd"]   # [128, 128] fp16 filled with 1/D
    mu_b = pst.tile([P, NB], F32, tag="st", name="mu_b")
    ms_b = pst.tile([P, NB], F32, tag="st", name="ms_b")

    hpre = []
    sqs = []
    for g in range(EGRP):
        ps_es = [pse.tile([P, NB], F32, tag="eig", name=f"eig{i}")
                 for i in range(EGS)]
        for jc in range(JC):
            we_t = wep_p.tile([P, EGS * P], F16, tag="wep")
            nc.sync.dma_start(out=we_t[:], in_=dram[f"we{blk}"][g, jc])
            for di in range(EGS):
                nc.tensor.matmul(
                    ps_es[di][:], we_t[:, di * P:(di + 1) * P], inter[jc][:],
                    start=(jc == 0), stop=(jc == JC - 1),
                )
        for di in range(EGS):
            do = g * EGS + di
            # fused: hpre = (psum + be) + residual, fp16 out, one DVE op
            hp = sb.tile([P, NB], F16, tag="hpre", bufs=24)
            nc.vector.scalar_tensor_tensor(hp[:], ps_es[di][:],
                                           be_sb[:, do:do + 1], acts[do][:],
                                           op0=ALU.add, op1=ALU.add)
            hpre.append(hp)
            sq = sb.tile([P, NB], F16, tag="sq", bufs=10)
            nc.scalar.activation(sq[:], hp[:], AF.Square)
            sqs.append(sq)
        if tail:
            hh = _tree(hpre[g * EGS:(g + 1) * EGS], "htree")
            sh = _tree(sqs[g * EGS:(g + 1) * EGS], "stree")
            nc.tensor.matmul(mu_b[:], ones_dd[:], hh[:],
                             start=(g == 0), stop=(g == EGRP - 1))
            nc.tensor.matmul(ms_b[:], ones_dd[:], sh[:],
                             start=(g == 0), stop=(g == EGRP - 1))

    if not tail:
        hsum = _tree(list(hpre), "htree")
        ssum = _tree(sqs, "stree")
        nc.tensor.matmul(mu_b[:], ones_dd[:], hsum[:], start=True, stop=True)
        nc.tensor.matmul(ms_b[:], ones_dd[:], ssum[:], start=True, stop=True)

    # full-width chain: var = ms - mu^2, rstd = sqrt(1/var)
    # (eps dropped: var ~ 2 >> eps, relative effect ~2.5e-6)
    # For the tail block the chain runs in column halves so the first
    # half's apply+head can start ~1.5us earlier.
    mu16 = sb.tile([P, NB], F16, tag="mu16", bufs=4)
    rstd16 = sb.tile([P, NB], F16, tag="rstd", bufs=4)
    col_splits = ([slice(0, NB // 2), slice(NB // 2, NB)] if tail
                  else [slice(0, NB)])
    for cs in col_splits:
        sqmu = sb.tile([P, NB], F32, tag="chain", bufs=6)
        nc.scalar.activation(sqmu[:, cs], mu_b[:, cs], AF.Square)
        nc.scalar.activation(mu16[:, cs], mu_b[:, cs], AF.Identity)
        var = sb.tile([P, NB], F32, tag="chain", bufs=6)
        nc.vector.scalar_tensor_tensor(var[:, cs], sqmu[:, cs], -1.0,
                                       ms_b[:, cs],
                                       op0=ALU.mult, op1=ALU.add)
        ivar = sb.tile([P, NB], F32, tag="chain", bufs=6)
        nc.vector.reciprocal_approx_fast(out=ivar[:, cs], in_=var[:, cs])
        nc.scalar.activation(rstd16[:, cs], ivar[:, cs], AF.Sqrt)
    return hpre, mu16, rstd16


def _emit_ln(nc, pools, blk, st, t, trivial_affine, cols=None):
    """LN apply: h[do] = (hpre[do]-mu)*rstd [*g[do]+b[do]] per chunk."""
    hpre, mu16, rstd16 = st
    sb, pp, cst = pools["sb"], pools["pp"], pools["const"]
    g_sb = cst[f"g{blk}"]
    bb_sb = cst[f"bb{blk}"]
    cs = slice(None) if cols is None else cols
    outs = []
    for do in range(DC):
        e = pp.tile([P, NB], F16, tag="lne")
        nc.vector.tensor_sub(e[:, cs], hpre[do][:, cs], mu16[:, cs])
        if trivial_affine:
            ho = sb.tile([P, NB], F16, tag=f"h{blk}", bufs=18)
            nc.vector.tensor_mul(ho[:, cs], e[:, cs], rstd16[:, cs])
        else:
            f = pp.tile([P, NB], F16, tag="lnf")
            nc.vector.tensor_mul(f[:, cs], e[:, cs], rstd16[:, cs])
            ho = sb.tile([P, NB], F16, tag=f"h{blk}", bufs=18)
            nc.scalar.activation(ho[:, cs], f[:, cs], AF.Identity,
                                 bias=bb_sb[:, do:do + 1],
                                 scale=g_sb[:, do:do + 1])
        outs.append(ho)
    return outs


def build_program(bc=BC, trivial_affine=True):
    """Build the per-core SPMD program. bc = rows per core."""
    nt = bc // NB
    nc = bacc.Bacc("TRN2", target_bir_lowering=False)

    dram = {
        "xT": nc.dram_tensor("xT", [D, bc], F16, kind="ExternalInput"),
        "wf": nc.dram_tensor("wf", [P, DC * OUT], F16, kind="ExternalInput"),
        "bf": nc.dram_tensor("bf", [OUT, 1], F32, kind="ExternalInput"),
        "outT": nc.dram_tensor("outT", [OUT, bc], F32, kind="ExternalOutput"),
    }
    for blk in (1, 2):
        dram[f"wr{blk}"] = nc.dram_tensor(f"wr{blk}", [JC, P, D], F16,
                                          kind="ExternalInput")
        dram[f"wl{blk}"] = nc.dram_tensor(f"wl{blk}", [JC, P, D], F16,
                                          kind="ExternalInput")
        dram[f"we{blk}"] = nc.dram_tensor(f"we{blk}", [EGRP, JC, P, EGS * P],
                                          F16, kind="ExternalInput")
        for nm, cols in ((f"br{blk}", JC), (f"bl{blk}", JC), (f"be{blk}", DC),
                         (f"g{blk}", DC), (f"bb{blk}", DC)):
            dram[nm] = nc.dram_tensor(nm, [P, cols], F32, kind="ExternalInput")

    with tile.TileContext(nc) as tc:
        with (
            tc.tile_pool(name="sb", bufs=2) as sb,
            tc.tile_pool(name="wp", bufs=6) as wp,
            tc.tile_pool(name="wep", bufs=8) as wep_p,
            tc.tile_pool(name="ip", bufs=36) as ip,
            tc.tile_pool(name="pp", bufs=4) as pp,
            tc.tile_pool(name="const", bufs=1) as cstp,
            tc.tile_pool(name="ps_rl", bufs=2, space="PSUM") as ps_rl,
            tc.tile_pool(name="ps_e", bufs=4, space="PSUM") as ps_e,
            tc.tile_pool(name="ps_st", bufs=2, space="PSUM") as ps_st,
        ):
            cst = {}
            for blk in (1, 2):
                for nm, cols in ((f"br{blk}", JC), (f"bl{blk}", JC),
                                 (f"be{blk}", DC), (f"g{blk}", DC),
                                 (f"bb{blk}", DC)):
                    cst[nm] = cstp.tile([P, cols], F32, tag=nm, name=nm)
                    nc.gpsimd.dma_start(out=cst[nm][:], in_=dram[nm][:])
            cst["ones_dd"] = cstp.tile([P, P], F16, tag="ones_dd",
                                       name="ones_dd")
            nc.vector.memset(cst["ones_dd"][:], 1.0 / D)
            cst["wf"] = cstp.tile([P, DC * OUT], F16, tag="wf", name="wf_sb")
            nc.gpsimd.dma_start(out=cst["wf"][:], in_=dram["wf"][:])
            cst["bf"] = cstp.tile([OUT, 1], F32, tag="bf", name="bf_sb")
            nc.gpsimd.dma_start(out=cst["bf"][:], in_=dram["bf"][:])

            pools = {
                "sb": sb, "wp": wp, "wep": wep_p, "ip": ip, "pp": pp,
                "const": cst, "ps_rl": ps_rl, "ps_e": ps_e, "ps_st": ps_st,
            }

            def emit_head(h2, t, cols=None):
                cs = slice(None) if cols is None else cols
                ncols = NB if cols is None else (cols.stop - cols.start)
                hd = ps_e.tile([P, NB], F32, tag="eig", name="hd")
                for dc in range(DC):
                    nc.tensor.matmul(
                        hd[0:OUT, cs], cst["wf"][:, dc * OUT:(dc + 1) * OUT],
                        h2[dc][:, cs], start=(dc == 0), stop=(dc == DC - 1),
                    )
                out_sb = sb.tile([OUT, NB], F32, tag="osb")
                nc.scalar.activation(out_sb[:, cs], hd[0:OUT, cs], AF.Identity,
                                     bias=cst["bf"][:])
                ob = t * NB + (0 if cols is None else cols.start)
                nc.gpsimd.dma_start(out=dram["outT"][:, ob:ob + ncols],
                                    in_=out_sb[:, cs])

            # warmup: a burst of throwaway matmuls fills the initial weight-DMA
            # wait and lifts the PE HAM clock gate to 8/8 before real work
            wm_l = cstp.tile([P, P], F16, tag="wm_l", name="wm_l")
            nc.vector.memset(wm_l[:], 0.0)
            wm_r = cstp.tile([P, NB], F16, tag="wm_r", name="wm_r")
            nc.vector.memset(wm_r[:], 0.0)
            for i in range(26):
                wps = ps_rl.tile([P, NB], F32, tag="rl", name=f"warm{i}")
                nc.tensor.matmul(wps[:], wm_l[:], wm_r[:],
                                 start=True, stop=True)

            # one-tile-lag slot order: every LN chain/apply and every head
            # hides under the NEXT slot's (independent) matmul stream.
            slots = [(1, 0), (1, 1)]
            for t in range(2, nt):
                slots += [(2, t - 2), (1, t)]
            slots += [(2, nt - 2), (2, nt - 1)]

            x_tiles = {}
            h1_tiles = {}
            st_pend = []        # [(blk, t, ln_state), ...] applies to emit
            for blk, t in slots:
                if blk == 1:
                    xs = []
                    for dc in range(DC):
                        xt = sb.tile([P, NB], F16, tag="xf16", bufs=16)
                        nc.scalar.dma_start(
                            out=xt[:],
                            in_=dram["xT"][dc * P:(dc + 1) * P,
                                           t * NB:(t + 1) * NB],
                        )
                        xs.append(xt)
                    x_tiles[t] = xs
                    acts = xs
                else:
                    acts = h1_tiles.pop(t)
                st = _emit_block_matmuls(nc, pools, dram, blk, acts, t,
                                         tail=(blk, t) == slots[-1])
                # emit deferred applies/heads AFTER this slot's matmuls so
                # the PE work (head) fills any residual chain gap and the
                # DVE work overlaps the next slot's stream.
                for pblk, pt, pst_ in st_pend:
                    h = _emit_ln(nc, pools, pblk, pst_, pt, trivial_affine)
                    if pblk == 1:
                        h1_tiles[pt] = h
                        x_tiles.pop(pt, None)
                    else:
                        emit_head(h, pt)
                st_pend = [(blk, t, st)]
            # final tail: last tile's LN2 + head, split into column halves
            # so the head matmuls overlap the second half's apply.
            (blk, t, st), = st_pend
            hmid = NB // 2
            h2a = _emit_ln(nc, pools, 2, st, t, trivial_affine,
                           cols=slice(0, hmid))
            emit_head(h2a, t, cols=slice(0, hmid))
            h2b = _emit_ln(nc, pools, 2, st, t, trivial_affine,
                           cols=slice(hmid, NB))
            emit_head(h2b, t, cols=slice(hmid, NB))
    nc.compile()
    return nc


def _f16(a):
    return np.ascontiguousarray(a.astype(np.float16))


def prep_inputs(inputs, bc=BC, ncores=NCORES):
    """Host-side shard + transpose + fp16 conversion. Returns in_maps."""
    f = {k: np.asarray(v, dtype=np.float32) for k, v in inputs.items()}

    shared = {}
    for blk in (1, 2):
        for side in ("r", "l"):
            w = f[f"w{side}{blk}"].reshape(HR, D)          # [j, d]
            panel = w.reshape(JC, P, DC, P).transpose(0, 3, 2, 1)
            shared[f"w{side}{blk}"] = _f16(panel.reshape(JC, P, D))
            shared[f"b{side}{blk}"] = np.ascontiguousarray(
                f[f"b{side}{blk}"].reshape(JC, P).T)        # [128, 32]
        weT = f[f"we{blk}"].T                               # [j, d_out]
        panel = weT.reshape(JC, P, EGRP, EGS * P).transpose(2, 0, 1, 3)
        shared[f"we{blk}"] = _f16(panel)                    # [g, jc, p, 512]
        shared[f"be{blk}"] = np.ascontiguousarray(
            f[f"be{blk}"].reshape(DC, P).T)                 # [128, 8]
        shared[f"g{blk}"] = np.ascontiguousarray(
            f[f"g{blk}"].reshape(DC, P).T)
        shared[f"bb{blk}"] = np.ascontiguousarray(
            f[f"b{blk}"].reshape(DC, P).T)
    shared["wf"] = _f16(f["wf"].T.reshape(DC, P, OUT).transpose(1, 0, 2)
                        .reshape(P, DC * OUT))              # [128, 80]
    shared["bf"] = np.ascontiguousarray(f["bf"].reshape(OUT, 1))

    x = f["x"]
    in_maps = []
    for c in range(ncores):
        m = dict(shared)
        m["xT"] = _f16(x[c * bc:(c + 1) * bc].T)            # [1024, bc]
        in_maps.append(m)
    return in_maps


def _affine_trivial(inputs):
    return all(
        np.allclose(np.asarray(inputs[k]), v, atol=1e-12)
        for k, v in (("g1", 1.0), ("g2", 1.0), ("b1", 0.0), ("b2", 0.0))
    )


_PROGRAM_CACHE = {}


def get_program(bc=BC, trivial_affine=True):
    key = (bc, trivial_affine)
    if key not in _PROGRAM_CACHE:
        _PROGRAM_CACHE[key] = build_program(bc, trivial_affine)
    return _PROGRAM_CACHE[key]


def kernel(**inputs):
    trivial = _affine_trivial(inputs)
    nc = get_program(BC, trivial)
    in_maps = prep_inputs(inputs, BC, NCORES)
    res = run_bass_kernel_spmd(nc, in_maps, core_ids=list(range(NCORES)))
    out = np.concatenate([res.results[c]["outT"] for c in range(NCORES)],
                         axis=1).T
    return np.ascontiguousarray(out.astype(np.float32))


if __name__ == "__main__":
    raise SystemExit("import kernel and call kernel(**inputs); see test.py")


# revision 13
# speedup vs baseline: 1.0067x; 1.0067x over previous
"""Trainium2 Bass kernel for DeepMultiBasisBilinearNet.

Strategy: pure data-parallel over the batch (16384/8 = 2048 rows per core).
All activations kept in transposed [D, B] layout on-device so that every
matmul contraction dim lands on SBUF partitions with zero on-device
transposes (host pre-transposes x and all weights). All matmuls run in
fp16 (fp32 PSUM accumulation). LayerNorm statistics are computed in
broadcast form: the 8 hpre chunks (and their squares) are tree-summed on
DVE, then ones/D stationary matmuls land the mean / mean-square on all
128 partitions at once, so the variance -> rstd chain and the two-op
apply run on full-width [128, NB] tiles with no PE involvement.

Block emissions are software-pipelined with a one-tile lag --
b1(0), b1(1), b2(0), b1(2), b2(1), ... -- so each layernorm chain+apply
(and each head) hides under an *independent* block's matmul stream; the
only exposed serial tail is the last tile's LN2+head.
"""

import sys

if "/opt/trn_rl_repo" not in sys.path:
    sys.path.insert(0, "/opt/trn_rl_repo")

import numpy as np

import concourse.bass as bass
import concourse.tile as tile
from concourse import bacc, mybir
from concourse.bass_utils import run_bass_kernel_spmd

F16 = mybir.dt.float16
F32 = mybir.dt.float32
AF = mybir.ActivationFunctionType
ALU = mybir.AluOpType

P = 128
B, D, H, R, OUT = 16384, 1024, 4, 1024, 10
HR = H * R                 # 4096
NCORES = 8
BC = B // NCORES           # 2048 rows per core
NB = 512                   # batch tile (matmul free dim, one PSUM bank)
DC = D // P                # 8 chunks of the model dim
JC = HR // P               # 32 chunks of the bilinear dim
EGRP = 2                   # eigen-projection dout groups (PSUM pressure)
EGS = DC // EGRP           # douts per group
LN_EPS = 1e-5


def _emit_block_matmuls(nc, pools, dram, blk, acts, t, tail=False):
    """Bilinear matmuls + residual + broadcast LN stats for batch tile t.

    acts: list of 8 SBUF fp16 tiles [128, NB] holding the input in [d, b]
    layout.  Returns (hpre chunk list, mu16, rstd16) for the LN apply.
    """
    sb, wp, wep_p, ip, pp, psr, pse, pst, cst = (
        pools["sb"], pools["wp"], pools["wep"], pools["ip"], pools["pp"],
        pools["ps_rl"], pools["ps_e"], pools["ps_st"], pools["const"],
    )
    br_sb = cst[f"br{blk}"]
    bl_sb = cst[f"bl{blk}"]
    be_sb = cst[f"be{blk}"]

    inter = []
    for jc in range(JC):
        wr_t = wp.tile([P, D], F16, tag="wrl")
        nc.sync.dma_start(out=wr_t[:], in_=dram[f"wr{blk}"][jc])
        wl_t = wp.tile([P, D], F16, tag="wrl")
        nc.sync.dma_start(out=wl_t[:], in_=dram[f"wl{blk}"][jc])

        ps_r = psr.tile([P, NB], F32, tag="rl")
        for dc in range(DC):
            nc.tensor.matmul(
                ps_r[:], wr_t[:, dc * P:(dc + 1) * P], acts[dc][:],
                start=(dc == 0), stop=(dc == DC - 1),
            )
        # evict right off PSUM immediately (ACT) so the bank frees fast
        tmp_r = pp.tile([P, NB], F16, tag="tmp_r")
        nc.scalar.activation(tmp_r[:], ps_r[:], AF.Identity,
                             bias=br_sb[:, jc:jc + 1])

        ps_l = psr.tile([P, NB], F32, tag="rl")
        for dc in range(DC):
            nc.tensor.matmul(
                ps_l[:], wl_t[:, dc * P:(dc + 1) * P], acts[dc][:],
                start=(dc == 0), stop=(dc == DC - 1),
            )
        tmp_l = pp.tile([P, NB], F16, tag="tmp_l")
        nc.scalar.activation(tmp_l[:], ps_l[:], AF.Identity,
                             bias=bl_sb[:, jc:jc + 1])

        it = ip.tile([P, NB], F16, tag="inter")
        nc.vector.tensor_mul(it[:], tmp_r[:], tmp_l[:])
        inter.append(it)

    # eigen projection: [D, NB] += weT[jc] @ inter[jc], in EGRP dout groups.
    # hpre chunks are pairwise tree-summed (values and squares) on DVE so
    # the broadcast LN stats need only one matmul per statistic.
    def _tree(nodes, tag):
        while len(nodes) > 1:
            nxt = []
            for i in range(0, len(nodes) - 1, 2):
                s = sb.tile([P, NB], F16, tag=tag, bufs=6)
                nc.vector.tensor_add(s[:], nodes[i][:], nodes[i + 1][:])
                nxt.append(s)
            if len(nodes) % 2:
                nxt.append(nodes[-1])
            nodes = nxt
        return nodes[0]

    # broadcast-form stats: ones/D stationary -> mean on all 128 partitions.
    # For the final (tail) block the stats matmuls are split into two
    # accumulation halves so half the reduction lands mid-eigen.
    ones_dd = cst["ones_dd"]   # [128, 128] fp16 filled with 1/D
    mu_b = pst.tile([P, NB], F32, tag="st", name="mu_b")
    ms_b = pst.tile([P, NB], F32, tag="st", name="ms_b")

    hpre = []
    sqs = []
    for g in range(EGRP):
        ps_es = [pse.tile([P, NB], F32, tag="eig", name=f"eig{i}")
                 for i in range(EGS)]
        for jc in range(JC):
            we_t = wep_p.tile([P, EGS * P], F16, tag="wep")
            nc.sync.dma_start(out=we_t[:], in_=dram[f"we{blk}"][g, jc])
            for di in range(EGS):
                nc.tensor.matmul(
                    ps_es[di][:], we_t[:, di * P:(di + 1) * P], inter[jc][:],
                    start=(jc == 0), stop=(jc == JC - 1),
                )
        for di in range(EGS):
            do = g * EGS + di
            # fused: hpre = (psum + be) + residual, fp16 out, one DVE op
            hp = sb.tile([P, NB], F16, tag="hpre", bufs=24)
            nc.vector.scalar_tensor_tensor(hp[:], ps_es[di][:],
                                           be_sb[:, do:do + 1], acts[do][:],
                                           op0=ALU.add, op1=ALU.add)
            hpre.append(hp)
            sq = sb.tile([P, NB], F16, tag="sq", bufs=10)
            nc.scalar.activation(sq[:], hp[:], AF.Square)
            sqs.append(sq)
        if tail:
            hh = _tree(hpre[g * EGS:(g + 1) * EGS], "htree")
            sh = _tree(sqs[g * EGS:(g + 1) * EGS], "stree")
            nc.tensor.matmul(mu_b[:], ones_dd[:], hh[:],
                             start=(g == 0), stop=(g == EGRP - 1))
            nc.tensor.matmul(ms_b[:], ones_dd[:], sh[:],
                             start=(g == 0), stop=(g == EGRP - 1))

    if not tail:
        hsum = _tree(list(hpre), "htree")
        ssum = _tree(sqs, "stree")
        nc.tensor.matmul(mu_b[:], ones_dd[:], hsum[:], start=True, stop=True)
        nc.tensor.matmul(ms_b[:], ones_dd[:], ssum[:], start=True, stop=True)

    # full-width chain: var = ms - mu^2, rstd = sqrt(1/var)
    # (eps dropped: var ~ 2 >> eps, relative effect ~2.5e-6)
    # For the tail block the chain runs in column halves so the first
    # half's apply+head can start ~1.5us earlier.
    mu16 = sb.tile([P, NB], F16, tag="mu16", bufs=4)
    rstd16 = sb.tile([P, NB], F16, tag="rstd", bufs=4)
    col_splits = ([slice(0, NB // 2), slice(NB // 2, NB)] if tail
                  else [slice(0, NB)])
    for cs in col_splits:
        sqmu = sb.tile([P, NB], F32, tag="chain", bufs=6)
        nc.scalar.activation(sqmu[:, cs], mu_b[:, cs], AF.Square)
        nc.scalar.activation(mu16[:, cs], mu_b[:, cs], AF.Identity)
        var = sb.tile([P, NB], F32, tag="chain", bufs=6)
        nc.vector.scalar_tensor_tensor(var[:, cs], sqmu[:, cs], -1.0,
                                       ms_b[:, cs],
                                       op0=ALU.mult, op1=ALU.add)
        ivar = sb.tile([P, NB], F32, tag="chain", bufs=6)
        nc.vector.reciprocal_approx_fast(out=ivar[:, cs], in_=var[:, cs])
        nc.scalar.activation(rstd16[:, cs], ivar[:, cs], AF.Sqrt)
    return hpre, mu16, rstd16


def _emit_ln(nc, pools, blk, st, t, trivial_affine, cols=None):
    """LN apply: h[do] = (hpre[do]-mu)*rstd [*g[do]+b[do]] per chunk."""
    hpre, mu16, rstd16 = st
    sb, pp, cst = pools["sb"], pools["pp"], pools["const"]
    g_sb = cst[f"g{blk}"]
    bb_sb = cst[f"bb{blk}"]
    cs = slice(None) if cols is None else cols
    outs = []
    for do in range(DC):
        e = pp.tile([P, NB], F16, tag="lne")
        nc.vector.tensor_sub(e[:, cs], hpre[do][:, cs], mu16[:, cs])
        if trivial_affine:
            ho = sb.tile([P, NB], F16, tag=f"h{blk}", bufs=18)
            nc.vector.tensor_mul(ho[:, cs], e[:, cs], rstd16[:, cs])
        else:
            f = pp.tile([P, NB], F16, tag="lnf")
            nc.vector.tensor_mul(f[:, cs], e[:, cs], rstd16[:, cs])
            ho = sb.tile([P, NB], F16, tag=f"h{blk}", bufs=18)
            nc.scalar.activation(ho[:, cs], f[:, cs], AF.Identity,
                                 bias=bb_sb[:, do:do + 1],
                                 scale=g_sb[:, do:do + 1])
        outs.append(ho)
    return outs


def build_program(bc=BC, trivial_affine=True):
    """Build the per-core SPMD program. bc = rows per core."""
    nt = bc // NB
    nc = bacc.Bacc("TRN2", target_bir_lowering=False)

    dram = {
        "xT": nc.dram_tensor("xT", [D, bc], F16, kind="ExternalInput"),
        "wf": nc.dram_tensor("wf", [P, DC * OUT], F16, kind="ExternalInput"),
        "bf": nc.dram_tensor("bf", [OUT, 1], F32, kind="ExternalInput"),
        "outT": nc.dram_tensor("outT", [OUT, bc], F32, kind="ExternalOutput"),
    }
    for blk in (1, 2):
        dram[f"wr{blk}"] = nc.dram_tensor(f"wr{blk}", [JC, P, D], F16,
                                          kind="ExternalInput")
        dram[f"wl{blk}"] = nc.dram_tensor(f"wl{blk}", [JC, P, D], F16,
                                          kind="ExternalInput")
        dram[f"we{blk}"] = nc.dram_tensor(f"we{blk}", [EGRP, JC, P, EGS * P],
                                          F16, kind="ExternalInput")
        for nm, cols in ((f"br{blk}", JC), (f"bl{blk}", JC), (f"be{blk}", DC),
                         (f"g{blk}", DC), (f"bb{blk}", DC)):
            dram[nm] = nc.dram_tensor(nm, [P, cols], F32, kind="ExternalInput")

    with tile.TileContext(nc) as tc:
        with (
            tc.tile_pool(name="sb", bufs=2) as sb,
            tc.tile_pool(name="wp", bufs=6) as wp,
            tc.tile_pool(name="wep", bufs=8) as wep_p,
            tc.tile_pool(name="ip", bufs=36) as ip,
            tc.tile_pool(name="pp", bufs=4) as pp,
            tc.tile_pool(name="const", bufs=1) as cstp,
            tc.tile_pool(name="ps_rl", bufs=2, space="PSUM") as ps_rl,
            tc.tile_pool(name="ps_e", bufs=4, space="PSUM") as ps_e,
            tc.tile_pool(name="ps_st", bufs=2, space="PSUM") as ps_st,
        ):
            cst = {}
            for blk in (1, 2):
                for nm, cols in ((f"br{blk}", JC), (f"bl{blk}", JC),
                                 (f"be{blk}", DC), (f"g{blk}", DC),
                                 (f"bb{blk}", DC)):
                    cst[nm] = cstp.tile([P, cols], F32, tag=nm, name=nm)
                    nc.gpsimd.dma_start(out=cst[nm][:], in_=dram[nm][:])
            cst["ones_dd"] = cstp.tile([P, P], F16, tag="ones_dd",
                                       name="ones_dd")
            nc.vector.memset(cst["ones_dd"][:], 1.0 / D)
            cst["wf"] = cstp.tile([P, DC * OUT], F16, tag="wf", name="wf_sb")
            nc.gpsimd.dma_start(out=cst["wf"][:], in_=dram["wf"][:])
            cst["bf"] = cstp.tile([OUT, 1], F32, tag="bf", name="bf_sb")
            nc.gpsimd.dma_start(out=cst["bf"][:], in_=dram["bf"][:])

            pools = {
                "sb": sb, "wp": wp, "wep": wep_p, "ip": ip, "pp": pp,
                "const": cst, "ps_rl": ps_rl, "ps_e": ps_e, "ps_st": ps_st,
            }

            def emit_head(h2, t, cols=None):
                cs = slice(None) if cols is None else cols
                ncols = NB if cols is None else (cols.stop - cols.start)
                hd = ps_e.tile([P, NB], F32, tag="eig", name="hd")
                for dc in range(DC):
                    nc.tensor.matmul(
                        hd[0:OUT, cs], cst["wf"][:, dc * OUT:(dc + 1) * OUT],
                        h2[dc][:, cs], start=(dc == 0), stop=(dc == DC - 1),
                    )
                out_sb = sb.tile([OUT, NB], F32, tag="osb")
                nc.scalar.activation(out_sb[:, cs], hd[0:OUT, cs], AF.Identity,
                                     bias=cst["bf"][:])
                ob = t * NB + (0 if cols is None else cols.start)
                nc.gpsimd.dma_start(out=dram["outT"][:, ob:ob + ncols],
                                    in_=out_sb[:, cs])

            # warmup: a burst of throwaway matmuls fills the initial weight-DMA
            # wait and lifts the PE HAM clock gate to 8/8 before real work
            wm_l = cstp.tile([P, P], F16, tag="wm_l", name="wm_l")
            nc.vector.memset(wm_l[:], 0.0)
            wm_r = cstp.tile([P, NB], F16, tag="wm_r", name="wm_r")
            nc.vector.memset(wm_r[:], 0.0)
            for i in range(22):
                wps = ps_rl.tile([P, NB], F32, tag="rl", name=f"warm{i}")
                nc.tensor.matmul(wps[:], wm_l[:], wm_r[:],
                                 start=True, stop=True)

            # one-tile-lag slot order: every LN chain/apply and every head
            # hides under the NEXT slot's (independent) matmul stream.
            slots = [(1, 0), (1, 1)]
            for t in range(2, nt):
                slots += [(2, t - 2), (1, t)]
            slots += [(2, nt - 2), (2, nt - 1)]

            x_tiles = {}
            h1_tiles = {}
            st_pend = []        # [(blk, t, ln_state), ...] applies to emit
            for blk, t in slots:
                if blk == 1:
                    xs = []
                    for dc in range(DC):
                        xt = sb.tile([P, NB], F16, tag="xf16", bufs=16)
                        nc.sync.dma_start(
                            out=xt[:],
                            in_=dram["xT"][dc * P:(dc + 1) * P,
                                           t * NB:(t + 1) * NB],
                        )
                        xs.append(xt)
                    x_tiles[t] = xs
                    acts = xs
                else:
                    acts = h1_tiles.pop(t)
                st = _emit_block_matmuls(nc, pools, dram, blk, acts, t,
                                         tail=(blk, t) == slots[-1])
                # emit deferred applies/heads AFTER this slot's matmuls so
                # the PE work (head) fills any residual chain gap and the
                # DVE work overlaps the next slot's stream.
                for pblk, pt, pst_ in st_pend:
                    h = _emit_ln(nc, pools, pblk, pst_, pt, trivial_affine)
                    if pblk == 1:
                        h1_tiles[pt] = h
                        x_tiles.pop(pt, None)
                    else:
                        emit_head(h, pt)
                st_pend = [(blk, t, st)]
            # final tail: last tile's LN2 + head, split into column halves
            # so the head matmuls overlap the second half's apply.
            (blk, t, st), = st_pend
            hmid = NB // 2
            h2a = _emit_ln(nc, pools, 2, st, t, trivial_affine,
                           cols=slice(0, hmid))
            emit_head(h2a, t, cols=slice(0, hmid))
            h2b = _emit_ln(nc, pools, 2, st, t, trivial_affine,
                           cols=slice(hmid, NB))
            emit_head(h2b, t, cols=slice(hmid, NB))
    nc.compile()
    return nc


def _f16(a):
    return np.ascontiguousarray(a.astype(np.float16))


def prep_inputs(inputs, bc=BC, ncores=NCORES):
    """Host-side shard + transpose + fp16 conversion. Returns in_maps."""
    f = {k: np.asarray(v, dtype=np.float32) for k, v in inputs.items()}

    shared = {}
    for blk in (1, 2):
        for side in ("r", "l"):
            w = f[f"w{side}{blk}"].reshape(HR, D)          # [j, d]
            panel = w.reshape(JC, P, DC, P).transpose(0, 3, 2, 1)
            shared[f"w{side}{blk}"] = _f16(panel.reshape(JC, P, D))
            shared[f"b{side}{blk}"] = np.ascontiguousarray(
                f[f"b{side}{blk}"].reshape(JC, P).T)        # [128, 32]
        weT = f[f"we{blk}"].T                               # [j, d_out]
        panel = weT.reshape(JC, P, EGRP, EGS * P).transpose(2, 0, 1, 3)
        shared[f"we{blk}"] = _f16(panel)                    # [g, jc, p, 512]
        shared[f"be{blk}"] = np.ascontiguousarray(
            f[f"be{blk}"].reshape(DC, P).T)                 # [128, 8]
        shared[f"g{blk}"] = np.ascontiguousarray(
            f[f"g{blk}"].reshape(DC, P).T)
        shared[f"bb{blk}"] = np.ascontiguousarray(
            f[f"b{blk}"].reshape(DC, P).T)
    shared["wf"] = _f16(f["wf"].T.reshape(DC, P, OUT).transpose(1, 0, 2)
                        .reshape(P, DC * OUT))              # [128, 80]
    shared["bf"] = np.ascontiguousarray(f["bf"].reshape(OUT, 1))

    x = f["x"]
    in_maps = []
    for c in range(ncores):
        m = dict(shared)
        m["xT"] = _f16(x[c * bc:(c + 1) * bc].T)            # [1024, bc]
        in_maps.append(m)
    return in_maps


def _affine_trivial(inputs):
    return all(
        np.allclose(np.asarray(inputs[k]), v, atol=1e-12)
        for k, v in (("g1", 1.0), ("g2", 1.0), ("b1", 0.0), ("b2", 0.0))
    )


_PROGRAM_CACHE = {}


def get_program(bc=BC, trivial_affine=True):
    key = (bc, trivial_affine)
    if key not in _PROGRAM_CACHE:
        _PROGRAM_CACHE[key] = build_program(bc, trivial_affine)
    return _PROGRAM_CACHE[key]


def kernel(**inputs):
    trivial = _affine_trivial(inputs)
    nc = get_program(BC, trivial)
    in_maps = prep_inputs(inputs, BC, NCORES)
    res = run_bass_kernel_spmd(nc, in_maps, core_ids=list(range(NCORES)))
    out = np.concatenate([res.results[c]["outT"] for c in range(NCORES)],
                         axis=1).T
    return np.ascontiguousarray(out.astype(np.float32))


if __name__ == "__main__":
    raise SystemExit("import kernel and call kernel(**inputs); see test.py")


# revision 14
# speedup vs baseline: 1.0071x; 1.0003x over previous
"""Trainium2 Bass kernel for DeepMultiBasisBilinearNet.

Strategy: pure data-parallel over the batch (16384/8 = 2048 rows per core).
All activations kept in transposed [D, B] layout on-device so that every
matmul contraction dim lands on SBUF partitions with zero on-device
transposes (host pre-transposes x and all weights). All matmuls run in
fp16 (fp32 PSUM accumulation). LayerNorm statistics are computed in
broadcast form: the 8 hpre chunks (and their squares) are tree-summed on
DVE, then ones/D stationary matmuls land the mean / mean-square on all
128 partitions at once, so the variance -> rstd chain and the two-op
apply run on full-width [128, NB] tiles with no PE involvement.

Block emissions are software-pipelined with a one-tile lag --
b1(0), b1(1), b2(0), b1(2), b2(1), ... -- so each layernorm chain+apply
(and each head) hides under an *independent* block's matmul stream; the
only exposed serial tail is the last tile's LN2+head.
"""

import sys

if "/opt/trn_rl_repo" not in sys.path:
    sys.path.insert(0, "/opt/trn_rl_repo")

import numpy as np

import concourse.bass as bass
import concourse.tile as tile
from concourse import bacc, mybir
from concourse.bass_utils import run_bass_kernel_spmd

F16 = mybir.dt.float16
F32 = mybir.dt.float32
AF = mybir.ActivationFunctionType
ALU = mybir.AluOpType

P = 128
B, D, H, R, OUT = 16384, 1024, 4, 1024, 10
HR = H * R                 # 4096
NCORES = 8
BC = B // NCORES           # 2048 rows per core
NB = 512                   # batch tile (matmul free dim, one PSUM bank)
DC = D // P                # 8 chunks of the model dim
JC = HR // P               # 32 chunks of the bilinear dim
EGRP = 2                   # eigen-projection dout groups (PSUM pressure)
EGS = DC // EGRP           # douts per group
LN_EPS = 1e-5


def _emit_block_matmuls(nc, pools, dram, blk, acts, t, tail=False):
    """Bilinear matmuls + residual + broadcast LN stats for batch tile t.

    acts: list of 8 SBUF fp16 tiles [128, NB] holding the input in [d, b]
    layout.  Returns (hpre chunk list, mu16, rstd16) for the LN apply.
    """
    sb, wp, wep_p, ip, pp, psr, pse, pst, cst = (
        pools["sb"], pools["wp"], pools["wep"], pools["ip"], pools["pp"],
        pools["ps_rl"], pools["ps_e"], pools["ps_st"], pools["const"],
    )
    br_sb = cst[f"br{blk}"]
    bl_sb = cst[f"bl{blk}"]
    be_sb = cst[f"be{blk}"]

    inter = []
    for jc in range(JC):
        wrl_t = wp.tile([P, 2 * D], F16, tag="wrl")
        nc.sync.dma_start(out=wrl_t[:], in_=dram[f"wrl{blk}"][jc])

        ps_r = psr.tile([P, NB], F32, tag="rl")
        for dc in range(DC):
            nc.tensor.matmul(
                ps_r[:], wrl_t[:, dc * P:(dc + 1) * P], acts[dc][:],
                start=(dc == 0), stop=(dc == DC - 1),
            )
        # evict right off PSUM immediately (ACT) so the bank frees fast
        tmp_r = pp.tile([P, NB], F16, tag="tmp_r")
        nc.scalar.activation(tmp_r[:], ps_r[:], AF.Identity,
                             bias=br_sb[:, jc:jc + 1])

        ps_l = psr.tile([P, NB], F32, tag="rl")
        for dc in range(DC):
            nc.tensor.matmul(
                ps_l[:], wrl_t[:, D + dc * P:D + (dc + 1) * P], acts[dc][:],
                start=(dc == 0), stop=(dc == DC - 1),
            )
        tmp_l = pp.tile([P, NB], F16, tag="tmp_l")
        nc.scalar.activation(tmp_l[:], ps_l[:], AF.Identity,
                             bias=bl_sb[:, jc:jc + 1])

        it = ip.tile([P, NB], F16, tag="inter")
        nc.vector.tensor_mul(it[:], tmp_r[:], tmp_l[:])
        inter.append(it)

    # eigen projection: [D, NB] += weT[jc] @ inter[jc], in EGRP dout groups.
    # hpre chunks are pairwise tree-summed (values and squares) on DVE so
    # the broadcast LN stats need only one matmul per statistic.
    def _tree(nodes, tag):
        while len(nodes) > 1:
            nxt = []
            for i in range(0, len(nodes) - 1, 2):
                s = sb.tile([P, NB], F16, tag=tag, bufs=6)
                nc.vector.tensor_add(s[:], nodes[i][:], nodes[i + 1][:])
                nxt.append(s)
            if len(nodes) % 2:
                nxt.append(nodes[-1])
            nodes = nxt
        return nodes[0]

    # broadcast-form stats: ones/D stationary -> mean on all 128 partitions.
    # For the final (tail) block the stats matmuls are split into two
    # accumulation halves so half the reduction lands mid-eigen.
    ones_dd = cst["ones_dd"]   # [128, 128] fp16 filled with 1/D
    mu_b = pst.tile([P, NB], F32, tag="st", name="mu_b")
    ms_b = pst.tile([P, NB], F32, tag="st", name="ms_b")

    hpre = []
    sqs = []
    for g in range(EGRP):
        ps_es = [pse.tile([P, NB], F32, tag="eig", name=f"eig{i}")
                 for i in range(EGS)]
        for jc2 in range(JC // 2):
            we_t = wep_p.tile([P, 2 * EGS * P], F16, tag="wep")
            nc.sync.dma_start(out=we_t[:], in_=dram[f"we{blk}"][g, jc2])
            for sub in range(2):
                jc = 2 * jc2 + sub
                for di in range(EGS):
                    nc.tensor.matmul(
                        ps_es[di][:],
                        we_t[:, (sub * EGS + di) * P:(sub * EGS + di + 1) * P],
                        inter[jc][:],
                        start=(jc == 0), stop=(jc == JC - 1),
                    )
        for di in range(EGS):
            do = g * EGS + di
            # fused: hpre = (psum + be) + residual, fp16 out, one DVE op
            hp = sb.tile([P, NB], F16, tag="hpre", bufs=24)
            nc.vector.scalar_tensor_tensor(hp[:], ps_es[di][:],
                                           be_sb[:, do:do + 1], acts[do][:],
                                           op0=ALU.add, op1=ALU.add)
            hpre.append(hp)
            sq = sb.tile([P, NB], F16, tag="sq", bufs=10)
            nc.scalar.activation(sq[:], hp[:], AF.Square)
            sqs.append(sq)
        if tail:
            hh = _tree(hpre[g * EGS:(g + 1) * EGS], "htree")
            sh = _tree(sqs[g * EGS:(g + 1) * EGS], "stree")
            nc.tensor.matmul(mu_b[:], ones_dd[:], hh[:],
                             start=(g == 0), stop=(g == EGRP - 1))
            nc.tensor.matmul(ms_b[:], ones_dd[:], sh[:],
                             start=(g == 0), stop=(g == EGRP - 1))

    if not tail:
        hsum = _tree(list(hpre), "htree")
        ssum = _tree(sqs, "stree")
        nc.tensor.matmul(mu_b[:], ones_dd[:], hsum[:], start=True, stop=True)
        nc.tensor.matmul(ms_b[:], ones_dd[:], ssum[:], start=True, stop=True)

    # full-width chain: var = ms - mu^2, rstd = sqrt(1/var)
    # (eps dropped: var ~ 2 >> eps, relative effect ~2.5e-6)
    # For the tail block the chain runs in column halves so the first
    # half's apply+head can start ~1.5us earlier.
    mu16 = sb.tile([P, NB], F16, tag="mu16", bufs=4)
    rstd16 = sb.tile([P, NB], F16, tag="rstd", bufs=4)
    col_splits = ([slice(0, NB // 2), slice(NB // 2, NB)] if tail
                  else [slice(0, NB)])
    for cs in col_splits:
        sqmu = sb.tile([P, NB], F32, tag="chain", bufs=6)
        nc.scalar.activation(sqmu[:, cs], mu_b[:, cs], AF.Square)
        nc.scalar.activation(mu16[:, cs], mu_b[:, cs], AF.Identity)
        var = sb.tile([P, NB], F32, tag="chain", bufs=6)
        nc.vector.scalar_tensor_tensor(var[:, cs], sqmu[:, cs], -1.0,
                                       ms_b[:, cs],
                                       op0=ALU.mult, op1=ALU.add)
        ivar = sb.tile([P, NB], F32, tag="chain", bufs=6)
        nc.vector.reciprocal_approx_fast(out=ivar[:, cs], in_=var[:, cs])
        nc.scalar.activation(rstd16[:, cs], ivar[:, cs], AF.Sqrt)
    return hpre, mu16, rstd16


def _emit_ln(nc, pools, blk, st, t, trivial_affine, cols=None):
    """LN apply: h[do] = (hpre[do]-mu)*rstd [*g[do]+b[do]] per chunk."""
    hpre, mu16, rstd16 = st
    sb, pp, cst = pools["sb"], pools["pp"], pools["const"]
    g_sb = cst[f"g{blk}"]
    bb_sb = cst[f"bb{blk}"]
    cs = slice(None) if cols is None else cols
    outs = []
    for do in range(DC):
        e = pp.tile([P, NB], F16, tag="lne")
        nc.vector.tensor_sub(e[:, cs], hpre[do][:, cs], mu16[:, cs])
        if trivial_affine:
            ho = sb.tile([P, NB], F16, tag=f"h{blk}", bufs=18)
            nc.vector.tensor_mul(ho[:, cs], e[:, cs], rstd16[:, cs])
        else:
            f = pp.tile([P, NB], F16, tag="lnf")
            nc.vector.tensor_mul(f[:, cs], e[:, cs], rstd16[:, cs])
            ho = sb.tile([P, NB], F16, tag=f"h{blk}", bufs=18)
            nc.scalar.activation(ho[:, cs], f[:, cs], AF.Identity,
                                 bias=bb_sb[:, do:do + 1],
                                 scale=g_sb[:, do:do + 1])
        outs.append(ho)
    return outs


def build_program(bc=BC, trivial_affine=True):
    """Build the per-core SPMD program. bc = rows per core."""
    nt = bc // NB
    nc = bacc.Bacc("TRN2", target_bir_lowering=False)

    dram = {
        "xT": nc.dram_tensor("xT", [P, DC, bc], F16, kind="ExternalInput"),
        "wf": nc.dram_tensor("wf", [P, DC * OUT], F16, kind="ExternalInput"),
        "bf": nc.dram_tensor("bf", [OUT, 1], F32, kind="ExternalInput"),
        "outT": nc.dram_tensor("outT", [OUT, bc], F32, kind="ExternalOutput"),
    }
    for blk in (1, 2):
        dram[f"wrl{blk}"] = nc.dram_tensor(f"wrl{blk}", [JC, P, 2 * D], F16,
                                           kind="ExternalInput")
        dram[f"we{blk}"] = nc.dram_tensor(f"we{blk}",
                                          [EGRP, JC // 2, P, 2 * EGS * P],
                                          F16, kind="ExternalInput")
        for nm, cols in ((f"br{blk}", JC), (f"bl{blk}", JC), (f"be{blk}", DC),
                         (f"g{blk}", DC), (f"bb{blk}", DC)):
            dram[nm] = nc.dram_tensor(nm, [P, cols], F32, kind="ExternalInput")

    with tile.TileContext(nc) as tc:
        with (
            tc.tile_pool(name="sb", bufs=2) as sb,
            tc.tile_pool(name="wp", bufs=4) as wp,
            tc.tile_pool(name="wep", bufs=6) as wep_p,
            tc.tile_pool(name="ip", bufs=36) as ip,
            tc.tile_pool(name="pp", bufs=4) as pp,
            tc.tile_pool(name="const", bufs=1) as cstp,
            tc.tile_pool(name="ps_rl", bufs=2, space="PSUM") as ps_rl,
            tc.tile_pool(name="ps_e", bufs=4, space="PSUM") as ps_e,
            tc.tile_pool(name="ps_st", bufs=2, space="PSUM") as ps_st,
        ):
            cst = {}
            for blk in (1, 2):
                for nm, cols in ((f"br{blk}", JC), (f"bl{blk}", JC),
                                 (f"be{blk}", DC), (f"g{blk}", DC),
                                 (f"bb{blk}", DC)):
                    cst[nm] = cstp.tile([P, cols], F32, tag=nm, name=nm)
                    nc.gpsimd.dma_start(out=cst[nm][:], in_=dram[nm][:])
            cst["ones_dd"] = cstp.tile([P, P], F16, tag="ones_dd",
                                       name="ones_dd")
            nc.vector.memset(cst["ones_dd"][:], 1.0 / D)
            cst["wf"] = cstp.tile([P, DC * OUT], F16, tag="wf", name="wf_sb")
            nc.gpsimd.dma_start(out=cst["wf"][:], in_=dram["wf"][:])
            cst["bf"] = cstp.tile([OUT, 1], F32, tag="bf", name="bf_sb")
            nc.gpsimd.dma_start(out=cst["bf"][:], in_=dram["bf"][:])

            pools = {
                "sb": sb, "wp": wp, "wep": wep_p, "ip": ip, "pp": pp,
                "const": cst, "ps_rl": ps_rl, "ps_e": ps_e, "ps_st": ps_st,
            }

            def emit_head(h2, t, cols=None):
                cs = slice(None) if cols is None else cols
                ncols = NB if cols is None else (cols.stop - cols.start)
                hd = ps_e.tile([P, NB], F32, tag="eig", name="hd")
                for dc in range(DC):
                    nc.tensor.matmul(
                        hd[0:OUT, cs], cst["wf"][:, dc * OUT:(dc + 1) * OUT],
                        h2[dc][:, cs], start=(dc == 0), stop=(dc == DC - 1),
                    )
                out_sb = sb.tile([OUT, NB], F32, tag="osb")
                nc.scalar.activation(out_sb[:, cs], hd[0:OUT, cs], AF.Identity,
                                     bias=cst["bf"][:])
                ob = t * NB + (0 if cols is None else cols.start)
                nc.gpsimd.dma_start(out=dram["outT"][:, ob:ob + ncols],
                                    in_=out_sb[:, cs])

            # warmup: a burst of throwaway matmuls fills the initial weight-DMA
            # wait and lifts the PE HAM clock gate to 8/8 before real work
            wm_l = cstp.tile([P, P], F16, tag="wm_l", name="wm_l")
            nc.vector.memset(wm_l[:], 0.0)
            wm_r = cstp.tile([P, NB], F16, tag="wm_r", name="wm_r")
            nc.vector.memset(wm_r[:], 0.0)
            for i in range(22):
                wps = ps_rl.tile([P, NB], F32, tag="rl", name=f"warm{i}")
                nc.tensor.matmul(wps[:], wm_l[:], wm_r[:],
                                 start=True, stop=True)

            # one-tile-lag slot order: every LN chain/apply and every head
            # hides under the NEXT slot's (independent) matmul stream.
            slots = [(1, 0), (1, 1)]
            for t in range(2, nt):
                slots += [(2, t - 2), (1, t)]
            slots += [(2, nt - 2), (2, nt - 1)]

            x_tiles = {}
            h1_tiles = {}
            st_pend = []        # [(blk, t, ln_state), ...] applies to emit
            for blk, t in slots:
                if blk == 1:
                    xt = sb.tile([P, DC, NB], F16, tag="xf16", bufs=2)
                    nc.sync.dma_start(
                        out=xt[:],
                        in_=dram["xT"][:, :, t * NB:(t + 1) * NB],
                    )
                    xs = [xt[:, dc, :] for dc in range(DC)]
                    x_tiles[t] = xs
                    acts = xs
                else:
                    acts = h1_tiles.pop(t)
                st = _emit_block_matmuls(nc, pools, dram, blk, acts, t,
                                         tail=(blk, t) == slots[-1])
                # emit deferred applies/heads AFTER this slot's matmuls so
                # the PE work (head) fills any residual chain gap and the
                # DVE work overlaps the next slot's stream.
                for pblk, pt, pst_ in st_pend:
                    h = _emit_ln(nc, pools, pblk, pst_, pt, trivial_affine)
                    if pblk == 1:
                        h1_tiles[pt] = h
                        x_tiles.pop(pt, None)
                    else:
                        emit_head(h, pt)
                st_pend = [(blk, t, st)]
            # final tail: last tile's LN2 + head, split into column halves
            # so the head matmuls overlap the second half's apply.
            (blk, t, st), = st_pend
            hmid = NB // 2
            h2a = _emit_ln(nc, pools, 2, st, t, trivial_affine,
                           cols=slice(0, hmid))
            emit_head(h2a, t, cols=slice(0, hmid))
            h2b = _emit_ln(nc, pools, 2, st, t, trivial_affine,
                           cols=slice(hmid, NB))
            emit_head(h2b, t, cols=slice(hmid, NB))
    nc.compile()
    return nc


def _f16(a):
    return np.ascontiguousarray(a.astype(np.float16))


def prep_inputs(inputs, bc=BC, ncores=NCORES):
    """Host-side shard + transpose + fp16 conversion. Returns in_maps."""
    f = {k: np.asarray(v, dtype=np.float32) for k, v in inputs.items()}

    shared = {}
    for blk in (1, 2):
        sides = {}
        for side in ("r", "l"):
            w = f[f"w{side}{blk}"].reshape(HR, D)          # [j, d]
            panel = w.reshape(JC, P, DC, P).transpose(0, 3, 2, 1)
            sides[side] = panel.reshape(JC, P, D)
            shared[f"b{side}{blk}"] = np.ascontiguousarray(
                f[f"b{side}{blk}"].reshape(JC, P).T)        # [128, 32]
        shared[f"wrl{blk}"] = _f16(
            np.concatenate([sides["r"], sides["l"]], axis=2))  # [jc, p, 2D]
        weT = f[f"we{blk}"].T                               # [j, d_out]
        panel = weT.reshape(JC, P, EGRP, EGS * P).transpose(2, 0, 1, 3)
        shared[f"we{blk}"] = _f16(
            panel.reshape(EGRP, JC // 2, 2 * P, EGS * P)
            .reshape(EGRP, JC // 2, 2, P, EGS * P)
            .transpose(0, 1, 3, 2, 4)
            .reshape(EGRP, JC // 2, P, 2 * EGS * P))        # [g, jc2, p, 1024]
        shared[f"be{blk}"] = np.ascontiguousarray(
            f[f"be{blk}"].reshape(DC, P).T)                 # [128, 8]
        shared[f"g{blk}"] = np.ascontiguousarray(
            f[f"g{blk}"].reshape(DC, P).T)
        shared[f"bb{blk}"] = np.ascontiguousarray(
            f[f"b{blk}"].reshape(DC, P).T)
    shared["wf"] = _f16(f["wf"].T.reshape(DC, P, OUT).transpose(1, 0, 2)
                        .reshape(P, DC * OUT))              # [128, 80]
    shared["bf"] = np.ascontiguousarray(f["bf"].reshape(OUT, 1))

    x = f["x"]
    in_maps = []
    for c in range(ncores):
        m = dict(shared)
        m["xT"] = _f16(x[c * bc:(c + 1) * bc].T.reshape(DC, P, bc)
                       .transpose(1, 0, 2))                 # [128, 8, bc]
        in_maps.append(m)
    return in_maps


def _affine_trivial(inputs):
    return all(
        np.allclose(np.asarray(inputs[k]), v, atol=1e-12)
        for k, v in (("g1", 1.0), ("g2", 1.0), ("b1", 0.0), ("b2", 0.0))
    )


_PROGRAM_CACHE = {}


def get_program(bc=BC, trivial_affine=True):
    key = (bc, trivial_affine)
    if key not in _PROGRAM_CACHE:
        _PROGRAM_CACHE[key] = build_program(bc, trivial_affine)
    return _PROGRAM_CACHE[key]


def kernel(**inputs):
    trivial = _affine_trivial(inputs)
    nc = get_program(BC, trivial)
    in_maps = prep_inputs(inputs, BC, NCORES)
    res = run_bass_kernel_spmd(nc, in_maps, core_ids=list(range(NCORES)))
    out = np.concatenate([res.results[c]["outT"] for c in range(NCORES)],
                         axis=1).T
    return np.ascontiguousarray(out.astype(np.float32))


if __name__ == "__main__":
    raise SystemExit("import kernel and call kernel(**inputs); see test.py")


# revision 15
# speedup vs baseline: 1.0075x; 1.0004x over previous
"""Trainium2 Bass kernel for DeepMultiBasisBilinearNet.

Strategy: pure data-parallel over the batch (16384/8 = 2048 rows per core).
All activations kept in transposed [D, B] layout on-device so that every
matmul contraction dim lands on SBUF partitions with zero on-device
transposes (host pre-transposes x and all weights). All matmuls run in
fp16 (fp32 PSUM accumulation). LayerNorm statistics are computed in
broadcast form: the 8 hpre chunks (and their squares) are tree-summed on
DVE, then ones/D stationary matmuls land the mean / mean-square on all
128 partitions at once, so the variance -> rstd chain and the two-op
apply run on full-width [128, NB] tiles with no PE involvement.

Block emissions are software-pipelined with a one-tile lag --
b1(0), b1(1), b2(0), b1(2), b2(1), ... -- so each layernorm chain+apply
(and each head) hides under an *independent* block's matmul stream; the
only exposed serial tail is the last tile's LN2+head.
"""

import sys

if "/opt/trn_rl_repo" not in sys.path:
    sys.path.insert(0, "/opt/trn_rl_repo")

import numpy as np

import concourse.bass as bass
import concourse.tile as tile
from concourse import bacc, mybir
from concourse.bass_utils import run_bass_kernel_spmd

F16 = mybir.dt.float16
F32 = mybir.dt.float32
AF = mybir.ActivationFunctionType
ALU = mybir.AluOpType

P = 128
B, D, H, R, OUT = 16384, 1024, 4, 1024, 10
HR = H * R                 # 4096
NCORES = 8
BC = B // NCORES           # 2048 rows per core
NB = 512                   # batch tile (matmul free dim, one PSUM bank)
DC = D // P                # 8 chunks of the model dim
JC = HR // P               # 32 chunks of the bilinear dim
EGRP = 2                   # eigen-projection dout groups (PSUM pressure)
EGS = DC // EGRP           # douts per group
LN_EPS = 1e-5


def _emit_block_matmuls(nc, pools, dram, blk, acts, t, tail=False):
    """Bilinear matmuls + residual + broadcast LN stats for batch tile t.

    acts: list of 8 SBUF fp16 tiles [128, NB] holding the input in [d, b]
    layout.  Returns (hpre chunk list, mu16, rstd16) for the LN apply.
    """
    sb, wp, wep_p, ip, pp, psr, pse, pst, cst = (
        pools["sb"], pools["wp"], pools["wep"], pools["ip"], pools["pp"],
        pools["ps_rl"], pools["ps_e"], pools["ps_st"], pools["const"],
    )
    br_sb = cst[f"br{blk}"]
    bl_sb = cst[f"bl{blk}"]
    be_sb = cst[f"be{blk}"]

    inter = []
    for jc in range(JC):
        wrl_t = wp.tile([P, 2 * D], F16, tag="wrl")
        nc.sync.dma_start(out=wrl_t[:], in_=dram[f"wrl{blk}"][jc])

        ps_r = psr.tile([P, NB], F32, tag="rl")
        for dc in range(DC):
            nc.tensor.matmul(
                ps_r[:], wrl_t[:, dc * P:(dc + 1) * P], acts[dc][:],
                start=(dc == 0), stop=(dc == DC - 1),
            )
        # evict right off PSUM immediately (ACT) so the bank frees fast
        tmp_r = pp.tile([P, NB], F16, tag="tmp_r")
        nc.scalar.activation(tmp_r[:], ps_r[:], AF.Identity,
                             bias=br_sb[:, jc:jc + 1])

        ps_l = psr.tile([P, NB], F32, tag="rl")
        for dc in range(DC):
            nc.tensor.matmul(
                ps_l[:], wrl_t[:, D + dc * P:D + (dc + 1) * P], acts[dc][:],
                start=(dc == 0), stop=(dc == DC - 1),
            )
        tmp_l = pp.tile([P, NB], F16, tag="tmp_l")
        nc.scalar.activation(tmp_l[:], ps_l[:], AF.Identity,
                             bias=bl_sb[:, jc:jc + 1])

        it = ip.tile([P, NB], F16, tag="inter")
        nc.vector.tensor_mul(it[:], tmp_r[:], tmp_l[:])
        inter.append(it)

    # eigen projection: [D, NB] += weT[jc] @ inter[jc], in EGRP dout groups.
    # hpre chunks are pairwise tree-summed (values and squares) on DVE so
    # the broadcast LN stats need only one matmul per statistic.
    def _tree(nodes, tag):
        while len(nodes) > 1:
            nxt = []
            for i in range(0, len(nodes) - 1, 2):
                s = sb.tile([P, NB], F16, tag=tag, bufs=6)
                nc.vector.tensor_add(s[:], nodes[i][:], nodes[i + 1][:])
                nxt.append(s)
            if len(nodes) % 2:
                nxt.append(nodes[-1])
            nodes = nxt
        return nodes[0]

    # broadcast-form stats: ones/D stationary -> mean on all 128 partitions.
    # For the final (tail) block the stats matmuls are split into two
    # accumulation halves so half the reduction lands mid-eigen.
    ones_dd = cst["ones_dd"]   # [128, 128] fp16 filled with 1/D
    mu_b = pst.tile([P, NB], F32, tag="st", name="mu_b")
    ms_b = pst.tile([P, NB], F32, tag="st", name="ms_b")

    hpre = []
    sqs = []
    for g in range(EGRP):
        ps_es = [pse.tile([P, NB], F32, tag="eig", name=f"eig{i}")
                 for i in range(EGS)]
        for jc2 in range(JC // 2):
            we_t = wep_p.tile([P, 2 * EGS * P], F16, tag="wep")
            nc.sync.dma_start(out=we_t[:], in_=dram[f"we{blk}"][g, jc2])
            for sub in range(2):
                jc = 2 * jc2 + sub
                for di in range(EGS):
                    nc.tensor.matmul(
                        ps_es[di][:],
                        we_t[:, (sub * EGS + di) * P:(sub * EGS + di + 1) * P],
                        inter[jc][:],
                        start=(jc == 0), stop=(jc == JC - 1),
                    )
        for di in range(EGS):
            do = g * EGS + di
            # fused: hpre = (psum + be) + residual, fp16 out, one DVE op
            hp = sb.tile([P, NB], F16, tag="hpre", bufs=24)
            nc.vector.scalar_tensor_tensor(hp[:], ps_es[di][:],
                                           be_sb[:, do:do + 1], acts[do][:],
                                           op0=ALU.add, op1=ALU.add)
            hpre.append(hp)
            sq = sb.tile([P, NB], F16, tag="sq", bufs=10)
            nc.scalar.activation(sq[:], hp[:], AF.Square)
            sqs.append(sq)
        if tail:
            hh = _tree(hpre[g * EGS:(g + 1) * EGS], "htree")
            sh = _tree(sqs[g * EGS:(g + 1) * EGS], "stree")
            nc.tensor.matmul(mu_b[:], ones_dd[:], hh[:],
                             start=(g == 0), stop=(g == EGRP - 1))
            nc.tensor.matmul(ms_b[:], ones_dd[:], sh[:],
                             start=(g == 0), stop=(g == EGRP - 1))

    if not tail:
        hsum = _tree(list(hpre), "htree")
        ssum = _tree(sqs, "stree")
        nc.tensor.matmul(mu_b[:], ones_dd[:], hsum[:], start=True, stop=True)
        nc.tensor.matmul(ms_b[:], ones_dd[:], ssum[:], start=True, stop=True)

    # full-width chain: var = ms - mu^2, rstd = sqrt(1/var)
    # (eps dropped: var ~ 2 >> eps, relative effect ~2.5e-6)
    # For the tail block the chain runs in column halves so the first
    # half's apply+head can start ~1.5us earlier.
    mu16 = sb.tile([P, NB], F16, tag="mu16", bufs=4)
    rstd16 = sb.tile([P, NB], F16, tag="rstd", bufs=4)
    col_splits = ([slice(0, NB // 2), slice(NB // 2, NB)] if tail
                  else [slice(0, NB)])
    for cs in col_splits:
        sqmu = sb.tile([P, NB], F32, tag="chain", bufs=6)
        nc.scalar.activation(sqmu[:, cs], mu_b[:, cs], AF.Square)
        nc.scalar.activation(mu16[:, cs], mu_b[:, cs], AF.Identity)
        var = sb.tile([P, NB], F32, tag="chain", bufs=6)
        nc.vector.scalar_tensor_tensor(var[:, cs], sqmu[:, cs], -1.0,
                                       ms_b[:, cs],
                                       op0=ALU.mult, op1=ALU.add)
        ivar = sb.tile([P, NB], F32, tag="chain", bufs=6)
        nc.vector.reciprocal_approx_fast(out=ivar[:, cs], in_=var[:, cs])
        nc.scalar.activation(rstd16[:, cs], ivar[:, cs], AF.Sqrt)
    return hpre, mu16, rstd16


def _emit_ln(nc, pools, blk, st, t, trivial_affine, cols=None):
    """LN apply: h[do] = (hpre[do]-mu)*rstd [*g[do]+b[do]] per chunk."""
    hpre, mu16, rstd16 = st
    sb, pp, cst = pools["sb"], pools["pp"], pools["const"]
    g_sb = cst[f"g{blk}"]
    bb_sb = cst[f"bb{blk}"]
    cs = slice(None) if cols is None else cols
    outs = []
    for do in range(DC):
        e = pp.tile([P, NB], F16, tag="lne")
        nc.vector.tensor_sub(e[:, cs], hpre[do][:, cs], mu16[:, cs])
        if trivial_affine:
            ho = sb.tile([P, NB], F16, tag=f"h{blk}", bufs=18)
            nc.vector.tensor_mul(ho[:, cs], e[:, cs], rstd16[:, cs])
        else:
            f = pp.tile([P, NB], F16, tag="lnf")
            nc.vector.tensor_mul(f[:, cs], e[:, cs], rstd16[:, cs])
            ho = sb.tile([P, NB], F16, tag=f"h{blk}", bufs=18)
            nc.scalar.activation(ho[:, cs], f[:, cs], AF.Identity,
                                 bias=bb_sb[:, do:do + 1],
                                 scale=g_sb[:, do:do + 1])
        outs.append(ho)
    return outs


def build_program(bc=BC, trivial_affine=True):
    """Build the per-core SPMD program. bc = rows per core."""
    nt = bc // NB
    nc = bacc.Bacc("TRN2", target_bir_lowering=False)

    dram = {
        "xT": nc.dram_tensor("xT", [P, DC, bc], F16, kind="ExternalInput"),
        "wf": nc.dram_tensor("wf", [P, DC * OUT], F16, kind="ExternalInput"),
        "bf": nc.dram_tensor("bf", [OUT, 1], F32, kind="ExternalInput"),
        "outT": nc.dram_tensor("outT", [OUT, bc], F32, kind="ExternalOutput"),
    }
    for blk in (1, 2):
        dram[f"wrl{blk}"] = nc.dram_tensor(f"wrl{blk}", [JC, P, 2 * D], F16,
                                           kind="ExternalInput")
        dram[f"we{blk}"] = nc.dram_tensor(f"we{blk}",
                                          [EGRP, JC // 2, P, 2 * EGS * P],
                                          F16, kind="ExternalInput")
        for nm, cols in ((f"br{blk}", JC), (f"bl{blk}", JC), (f"be{blk}", DC),
                         (f"g{blk}", DC), (f"bb{blk}", DC)):
            dram[nm] = nc.dram_tensor(nm, [P, cols], F32, kind="ExternalInput")

    with tile.TileContext(nc) as tc:
        with (
            tc.tile_pool(name="sb", bufs=2) as sb,
            tc.tile_pool(name="wp", bufs=4) as wp,
            tc.tile_pool(name="wep", bufs=6) as wep_p,
            tc.tile_pool(name="ip", bufs=36) as ip,
            tc.tile_pool(name="pp", bufs=4) as pp,
            tc.tile_pool(name="const", bufs=1) as cstp,
            tc.tile_pool(name="ps_rl", bufs=2, space="PSUM") as ps_rl,
            tc.tile_pool(name="ps_e", bufs=4, space="PSUM") as ps_e,
            tc.tile_pool(name="ps_st", bufs=2, space="PSUM") as ps_st,
        ):
            cst = {}
            for blk in (1, 2):
                for nm, cols in ((f"br{blk}", JC), (f"bl{blk}", JC),
                                 (f"be{blk}", DC), (f"g{blk}", DC),
                                 (f"bb{blk}", DC)):
                    cst[nm] = cstp.tile([P, cols], F32, tag=nm, name=nm)
                    nc.gpsimd.dma_start(out=cst[nm][:], in_=dram[nm][:])
            cst["ones_dd"] = cstp.tile([P, P], F16, tag="ones_dd",
                                       name="ones_dd")
            nc.vector.memset(cst["ones_dd"][:], 1.0 / D)
            cst["wf"] = cstp.tile([P, DC * OUT], F16, tag="wf", name="wf_sb")
            nc.gpsimd.dma_start(out=cst["wf"][:], in_=dram["wf"][:])
            cst["bf"] = cstp.tile([OUT, 1], F32, tag="bf", name="bf_sb")
            nc.gpsimd.dma_start(out=cst["bf"][:], in_=dram["bf"][:])

            pools = {
                "sb": sb, "wp": wp, "wep": wep_p, "ip": ip, "pp": pp,
                "const": cst, "ps_rl": ps_rl, "ps_e": ps_e, "ps_st": ps_st,
            }

            def emit_head(h2, t, cols=None):
                cs = slice(None) if cols is None else cols
                ncols = NB if cols is None else (cols.stop - cols.start)
                hd = ps_e.tile([P, NB], F32, tag="eig", name="hd")
                for dc in range(DC):
                    nc.tensor.matmul(
                        hd[0:OUT, cs], cst["wf"][:, dc * OUT:(dc + 1) * OUT],
                        h2[dc][:, cs], start=(dc == 0), stop=(dc == DC - 1),
                    )
                out_sb = sb.tile([OUT, NB], F32, tag="osb")
                nc.scalar.activation(out_sb[:, cs], hd[0:OUT, cs], AF.Identity,
                                     bias=cst["bf"][:])
                ob = t * NB + (0 if cols is None else cols.start)
                nc.gpsimd.dma_start(out=dram["outT"][:, ob:ob + ncols],
                                    in_=out_sb[:, cs])

            # warmup: a burst of throwaway matmuls fills the initial weight-DMA
            # wait and lifts the PE HAM clock gate to 8/8 before real work
            wm_l = cstp.tile([P, P], F16, tag="wm_l", name="wm_l")
            nc.vector.memset(wm_l[:], 0.0)
            wm_r = cstp.tile([P, NB], F16, tag="wm_r", name="wm_r")
            nc.vector.memset(wm_r[:], 0.0)
            for i in range(28):
                wps = ps_rl.tile([P, NB], F32, tag="rl", name=f"warm{i}")
                nc.tensor.matmul(wps[:], wm_l[:], wm_r[:],
                                 start=True, stop=True)

            # one-tile-lag slot order: every LN chain/apply and every head
            # hides under the NEXT slot's (independent) matmul stream.
            slots = [(1, 0), (1, 1)]
            for t in range(2, nt):
                slots += [(2, t - 2), (1, t)]
            slots += [(2, nt - 2), (2, nt - 1)]

            x_tiles = {}
            h1_tiles = {}
            st_pend = []        # [(blk, t, ln_state), ...] applies to emit
            for blk, t in slots:
                if blk == 1:
                    xt = sb.tile([P, DC, NB], F16, tag="xf16", bufs=2)
                    nc.sync.dma_start(
                        out=xt[:],
                        in_=dram["xT"][:, :, t * NB:(t + 1) * NB],
                    )
                    xs = [xt[:, dc, :] for dc in range(DC)]
                    x_tiles[t] = xs
                    acts = xs
                else:
                    acts = h1_tiles.pop(t)
                st = _emit_block_matmuls(nc, pools, dram, blk, acts, t,
                                         tail=(blk, t) == slots[-1])
                # emit deferred applies/heads AFTER this slot's matmuls so
                # the PE work (head) fills any residual chain gap and the
                # DVE work overlaps the next slot's stream.
                for pblk, pt, pst_ in st_pend:
                    h = _emit_ln(nc, pools, pblk, pst_, pt, trivial_affine)
                    if pblk == 1:
                        h1_tiles[pt] = h
                        x_tiles.pop(pt, None)
                    else:
                        emit_head(h, pt)
                st_pend = [(blk, t, st)]
            # final tail: last tile's LN2 + head, split into column halves
            # so the head matmuls overlap the second half's apply.
            (blk, t, st), = st_pend
            hmid = NB // 2
            h2a = _emit_ln(nc, pools, 2, st, t, trivial_affine,
                           cols=slice(0, hmid))
            emit_head(h2a, t, cols=slice(0, hmid))
            h2b = _emit_ln(nc, pools, 2, st, t, trivial_affine,
                           cols=slice(hmid, NB))
            emit_head(h2b, t, cols=slice(hmid, NB))
    nc.compile()
    return nc


def _f16(a):
    return np.ascontiguousarray(a.astype(np.float16))


def prep_inputs(inputs, bc=BC, ncores=NCORES):
    """Host-side shard + transpose + fp16 conversion. Returns in_maps."""
    f = {k: np.asarray(v, dtype=np.float32) for k, v in inputs.items()}

    shared = {}
    for blk in (1, 2):
        sides = {}
        for side in ("r", "l"):
            w = f[f"w{side}{blk}"].reshape(HR, D)          # [j, d]
            panel = w.reshape(JC, P, DC, P).transpose(0, 3, 2, 1)
            sides[side] = panel.reshape(JC, P, D)
            shared[f"b{side}{blk}"] = np.ascontiguousarray(
                f[f"b{side}{blk}"].reshape(JC, P).T)        # [128, 32]
        shared[f"wrl{blk}"] = _f16(
            np.concatenate([sides["r"], sides["l"]], axis=2))  # [jc, p, 2D]
        weT = f[f"we{blk}"].T                               # [j, d_out]
        panel = weT.reshape(JC, P, EGRP, EGS * P).transpose(2, 0, 1, 3)
        shared[f"we{blk}"] = _f16(
            panel.reshape(EGRP, JC // 2, 2 * P, EGS * P)
            .reshape(EGRP, JC // 2, 2, P, EGS * P)
            .transpose(0, 1, 3, 2, 4)
            .reshape(EGRP, JC // 2, P, 2 * EGS * P))        # [g, jc2, p, 1024]
        shared[f"be{blk}"] = np.ascontiguousarray(
            f[f"be{blk}"].reshape(DC, P).T)                 # [128, 8]
        shared[f"g{blk}"] = np.ascontiguousarray(
            f[f"g{blk}"].reshape(DC, P).T)
        shared[f"bb{blk}"] = np.ascontiguousarray(
            f[f"b{blk}"].reshape(DC, P).T)
    shared["wf"] = _f16(f["wf"].T.reshape(DC, P, OUT).transpose(1, 0, 2)
                        .reshape(P, DC * OUT))              # [128, 80]
    shared["bf"] = np.ascontiguousarray(f["bf"].reshape(OUT, 1))

    x = f["x"]
    in_maps = []
    for c in range(ncores):
        m = dict(shared)
        m["xT"] = _f16(x[c * bc:(c + 1) * bc].T.reshape(DC, P, bc)
                       .transpose(1, 0, 2))                 # [128, 8, bc]
        in_maps.append(m)
    return in_maps


def _affine_trivial(inputs):
    return all(
        np.allclose(np.asarray(inputs[k]), v, atol=1e-12)
        for k, v in (("g1", 1.0), ("g2", 1.0), ("b1", 0.0), ("b2", 0.0))
    )


_PROGRAM_CACHE = {}


def get_program(bc=BC, trivial_affine=True):
    key = (bc, trivial_affine)
    if key not in _PROGRAM_CACHE:
        _PROGRAM_CACHE[key] = build_program(bc, trivial_affine)
    return _PROGRAM_CACHE[key]


def kernel(**inputs):
    trivial = _affine_trivial(inputs)
    nc = get_program(BC, trivial)
    in_maps = prep_inputs(inputs, BC, NCORES)
    res = run_bass_kernel_spmd(nc, in_maps, core_ids=list(range(NCORES)))
    out = np.concatenate([res.results[c]["outT"] for c in range(NCORES)],
                         axis=1).T
    return np.ascontiguousarray(out.astype(np.float32))


if __name__ == "__main__":
    raise SystemExit("import kernel and call kernel(**inputs); see test.py")
